# revision 29
# baseline (speedup 1.0000x reference)
"""Block-mode DP kernel with a pre-staged SWDGE scatter output.

Same DP as kernel.py; the output DMA is a dma_scatter_add(prepare_only)
whose descriptors are generated mid-loop on the idle Pool engine; the tail
trigger_dma pays only transfer + DMA-sem, skipping the HWDGE+DGE chain.
Nothing in-program waits on the scatter's completion sem (a waiter would
recreate the barrier cycle that deadlocks the cost model); the sim total
still includes the transfer + sem events, and the executor applies the
data move at trigger time, before program end. All sems are cleared at
program start on the idle ACT engine (they persist across executions).
"""

import sys

import numpy as np

sys.path.insert(0, "/opt/trn_rl_repo")

import concourse.bacc as bacc
import concourse.mybir as mybir
from concourse.bass_utils import run_bass_kernel_spmd

P = 128
Q = 2
H = 64
W = 64
QW = Q * W
STEPS = 32
NB_CORE = P * Q
N_CORES = 8
BIG = 3.0e4
BIAS = 16.0
ZROW = 192     # per-chain stride in the padded state row (384 f16 = 768B)
F16 = mybir.dt.float16
I16 = mybir.dt.int16
MIN = mybir.AluOpType.min
ADD = mybir.AluOpType.add

CHUNKS = [(0, 3), (3, 8), (8, 18), (18, 56), (56, 64)]

_CACHE = {}


def _build():
    nc = bacc.Bacc("TRN2", debug=False, target_bir_lowering=False,
                   num_devices=N_CORES)
    img_d = nc.dram_tensor("images", [P, H, QW], F16, kind="ExternalInput")
    idx_d = nc.dram_tensor("idxs", [P, 8], I16, kind="ExternalInput")
    out_d = nc.dram_tensor("out", [P, 2 * ZROW], F16, kind="ExternalOutput")

    need = {}
    for k, (a, b) in enumerate(CHUNKS, start=1):
        for row in range(a, b):
            need[("F" if row % 2 == 0 else "B", row // 2)] = 16 * k
    NDMA = len(CHUNKS) + 2   # + idxs + zero-fill

    with (nc.Block() as block,
          nc.sbuf_tensor("imgT", [P, H, QW], F16) as imgT,
          nc.sbuf_tensor("zq", [P, 2 * ZROW], F16) as zq,
          nc.sbuf_tensor("mF", [P, QW], F16) as mF_t,
          nc.sbuf_tensor("mB", [P, QW], F16) as mB_t,
          nc.sbuf_tensor("idxsT", [P, 8], I16) as idxsT,
          nc.sbuf_tensor("zeroT", [P, 2 * ZROW], F16) as zeroT,
          nc.semaphore("io") as io,
          nc.semaphore("dv") as dv,
          nc.semaphore("pr") as pr,
          nc.semaphore("dsem") as dsem):

        @block.scalar
        def _(act):
            # sems persist across executions of the loaded program
            act.sem_clear(io)
            act.sem_clear(dv)
            act.sem_clear(pr)
            act.sem_clear(dsem)

        @block.sync
        def _(sync):
            for a, b in CHUNKS:
                sync.dma_start(imgT[:, a:b, :], img_d[:, a:b, :]).then_inc(io, 16)
            sync.dma_start(idxsT[:], idx_d[:]).then_inc(io, 16)
            # scatter-add accumulates; dst must start at zero
            sync.dma_start(out_d[:], zeroT[:]).then_inc(io, 16)

        @block.gpsimd
        def _(pool):
            # descriptor prep mid-loop on the idle Pool engine; the zq read
            # happens at trigger time
            pool.wait_ge(io, 16 * NDMA)
            pool.dma_scatter_add(
                out_d[:], zq[:].rearrange("p (a b) -> p a b", a=1), idxsT[:],
                P, P, 2 * ZROW, prepare_only=True, sem=dsem).then_inc(pr, 1)
            pool.wait_ge(pr, 1)       # descriptors committed to the ring
            pool.wait_ge(dv, 1)       # zq final (last scan's then_inc)
            pool.trigger_dma(count=1)

        @block.vector
        def _(dve):
            m = {"F": mF_t, "B": mB_t}
            zoff = {"F": 0, "B": ZROW}
            dve.memset(zq[:], 0.0)            # pads + state, covers garbage
            dve.memset(zeroT[:], 0.0)
            for d in "FB":
                dve.memset(zq[:, zoff[d]:zoff[d] + 1], BIG)
                dve.memset(m[d][:], BIG)
                dve.memset(m[d][:, 0:1], BIAS)
                dve.memset(m[d][:, W:W + 1], 0.0)

            state = {"th": 0}

            def gate(d, r):
                th = need[(d, r)]
                if th > state["th"]:
                    dve.wait_ge(io, th)
                    state["th"] = th

            def sstep(d, r):
                row = imgT[:, 2 * r, :] if d == "F" else imgT[:, 2 * r + 1, ::-1]
                o = zoff[d]
                return dve.tensor_tensor_scan(
                    out=zq[:, o + 1:o + 1 + QW], data0=m[d][:], data1=row,
                    initial=BIG, op0=MIN, op1=ADD)

            def mstep(d):
                o = zoff[d]
                dve.tensor_tensor(out=m[d][:], in0=zq[:, o + 1:o + 1 + QW],
                                  in1=zq[:, o:o + QW], op=MIN)

            last = None
            for r in range(STEPS):
                gate("F", r)
                sstep("F", r)
                gate("B", r)
                last = sstep("B", r)
                if r + 1 < STEPS:
                    mstep("F")
                    mstep("B")
            last.then_inc(dv, 1)

    nc.compile()
    return nc


def get_nc():
    if "nc" not in _CACHE:
        _CACHE["nc"] = _build()
    return _CACHE["nc"]


_ROW_ORD = np.empty(H, dtype=np.int64)
_ROW_ORD[0::2] = np.arange(H // 2)
_ROW_ORD[1::2] = H - 1 - np.arange(H // 2)

_IDXS = (np.arange(8, dtype=np.int16)[None, :] * 16
         + (np.arange(P, dtype=np.int16) % 16)[:, None]).copy()


def kernel(images: np.ndarray, **run_kwargs) -> np.ndarray:
    B = images.shape[0]
    assert images.shape == (B, H, W) and B == N_CORES * NB_CORE
    images = np.ascontiguousarray(images, dtype=np.float32)
    img16 = images.astype(np.float16)
    in_maps = []
    for c in range(N_CORES):
        shard = img16[c * NB_CORE:(c + 1) * NB_CORE]
        s = shard.reshape(Q, P, H, W).transpose(1, 2, 0, 3)[:, _ROW_ORD]
        in_maps.append({"images": np.ascontiguousarray(s).reshape(P, H, QW),
                        "idxs": _IDXS})
    nc = get_nc()
    res = run_bass_kernel_spmd(nc, in_maps, core_ids=list(range(N_CORES)),
                               **run_kwargs)
    out = np.empty((B,), dtype=np.float32)
    for c in range(N_CORES):
        zz = res.results[c]["out"].astype(np.float32)
        zf = zz[:, 1:1 + QW].reshape(P, Q, W)
        zb = zz[:, ZROW + 1:ZROW + 1 + QW].reshape(P, Q, W)[:, ::-1, ::-1]
        cand = zf + zb
        np.minimum(cand[:, :, :W - 1], zf[:, :, :W - 1] + zb[:, :, 1:],
                   out=cand[:, :, :W - 1])
        v = cand.min(axis=2) - BIAS
        out[c * NB_CORE:(c + 1) * NB_CORE] = v.T.reshape(-1)
    out -= 0.5 * (images[:, 0, 0] + images[:, H - 1, W - 1])
    if run_kwargs:
        return out, res
    return out
